# revision 13
# baseline (speedup 1.0000x reference)
"""Weighted 2D Gaussian KDE on 8 Trainium2 NeuronCores (Bass/Tile), v3.

out[b,l] = sum_n w[n] * exp(-||x[b,l] - data[n]||^2 / sigma),  sigma = 3.

Grid-quadrature factorization (v2): with a uniform grid u_j (spacing h,
a = 2/sigma, F = h*sqrt(2a/pi)),
    out[c] = q0(x_c)^T (F^2 P1 diag(w) P0^T) q1(x_c),
    P_d[j,n] = exp(-a(u_j - d_nd)^2),  q_d[j,c] = exp(-a(u_j - x_cd)^2).

v3 redesign: stage B packs MULTIPLE locations per exp column by windowing
the grid support of each location (gaussian decay => ~10 nodes matter per
dim). Locations are 2D-sorted into tiles sharing a window pair (o0, o1):
  mode 4: col = 4 locs x [win0(16) | win1(16)] stacked in 32-row bands
  mode 2: col = 2 locs x [win0(16) | dim1 full(48)]
  mode 1: col = 1 loc  x [dim0(48) | pad | dim1(48) | pad]  (x tails)
This cuts stage-B exp/mul/matmul free-dim cost ~3.3x (the critical-path
Activation engine runs ~1 col per 2-4 locations instead of 1 per loc).
The tile schedule is SHARED across cores (SPMD: one program) by forming
tiles on aligned sorted-rank ranges and windowing the union box over all
8 cores. argB lhsT tiles are host-built block-diagonal coefficient
matrices (one per distinct (mode,o0,o1)); the windowed-M T-matmul lhsT
tiles are built on device from m2 via banded-diagonal selector matmuls
(selM const sliced per (o1,sub)) + one batched PSUM->SBUF copy.

Sharding: locations (B*L = 131072) split contiguously across 8 cores
(16384 each); data/weights replicated; moment matrix computed
redundantly on every core (collectives cost >=15us fixed).
"""

import os
import numpy as np
import ml_dtypes

import concourse.bass as bass
import concourse.tile as tile
from concourse import bacc
from concourse import mybir
from concourse import bass_utils

BF = ml_dtypes.bfloat16

# ---- problem constants (hardcoded per spec) ----
B, L, D = 2, 65536, 2
NPTS = 16384
NCORES = 8
NLOC = B * L              # 131072 locations
NSH = NLOC // NCORES      # 16384 per core
SIGMA = 3.0
A = 2.0 / SIGMA
PADG = 3.0                # grid extension beyond data/location range
NG = 64                   # partition stride per dim for stage A layout
NGR = 48                  # real grid node count; spacing adapts to range
HMIN = 0.75
W = 16                    # stage-B window nodes per dim (modes 4/2)
REACH = 3.0               # min gaussian reach beyond a tile's box
QUANT = 2                 # window offset quantization (node units)

KD = 16                   # featd rows
NCHUNK = NPTS // 128      # 128 data chunks
AGRP = 16                 # stage-A chunks per exp batch
W0A = 20                  # stage-A dim0 grid window (nodes) per chunk
RPADA = 4.0               # stage-A window reach beyond a chunk's d0 range
EXPB = 4                  # stage-B blocks per exp batch (128 cols each)

F32 = mybir.dt.float32
BF16 = mybir.dt.bfloat16
AF = mybir.ActivationFunctionType


def _build_core_program(nc: bass.Bass, key):
    w0a, o0s, blocks, ndist, dmap, panels = key
    # blocks: tuple of (mode, dist_id); dmap: (mode, o0, o1, panel_idx) per
    # dist; panels: tuple of o1 values (-1 = full 48-row diagonal, mode-2)
    NBLK = len(blocks)
    TCOLS = NBLK * 128
    NPAN = len(panels)

    featd = nc.dram_tensor("featd", [128, 2048], BF16, kind="ExternalInput").ap()
    ga = nc.dram_tensor("ga", [128, 8 * NG], BF16, kind="ExternalInput").ap()
    featx = nc.dram_tensor("featx", [128, TCOLS], BF16, kind="ExternalInput").ap()
    gblk = nc.dram_tensor("gblk", [128, ndist * 128], BF16, kind="ExternalInput").ap()
    selm = nc.dram_tensor("selm", [48, NPAN * 224], BF16, kind="ExternalInput").ap()
    onesm = nc.dram_tensor("onesm", [128, 8], BF16, kind="ExternalInput").ap()
    out = nc.dram_tensor("out", [NSH], F32, kind="ExternalOutput").ap()

    with tile.TileContext(nc) as tc:
        with (
            tc.tile_pool(name="const", bufs=1) as const,
            tc.tile_pool(name="sbA", bufs=2) as sbA,
            tc.tile_pool(name="sbQ", bufs=6) as sbQ,
            tc.tile_pool(name="sbR", bufs=4) as sbR,
        ):
            # featd halves + ga first on separate DGE queues so stage A can
            # start ASAP. Nothing on the Activation queue (sequencer must be
            # free for the first exp).
            featd_sb = const.tile([128, 2048], BF16)
            Q4 = 2048 // 4
            nc.sync.dma_start(out=featd_sb[:, 0:Q4], in_=featd[:, 0:Q4])
            ga_sb = const.tile([128, 8 * NG], BF16)
            nc.sync.dma_start(out=ga_sb, in_=ga)
            for qi, qeng in [(1, nc.gpsimd), (2, nc.sync), (3, nc.gpsimd)]:
                qeng.dma_start(
                    out=featd_sb[:, qi * Q4 : (qi + 1) * Q4],
                    in_=featd[:, qi * Q4 : (qi + 1) * Q4],
                )
            selm_sb = const.tile([128, NPAN * 224], BF16)
            nc.sync.dma_start(out=selm_sb[64:112, :], in_=selm)
            onesm_sb = const.tile([128, 8], BF16)
            nc.sync.dma_start(out=onesm_sb, in_=onesm)
            featx_sb = const.tile([128, TCOLS], BF16)
            FQ4 = TCOLS // 4
            for qi, qeng in [(0, nc.sync), (1, nc.gpsimd), (2, nc.sync),
                             (3, nc.gpsimd)]:
                qeng.dma_start(
                    out=featx_sb[:, qi * FQ4 : (qi + 1) * FQ4],
                    in_=featx[:, qi * FQ4 : (qi + 1) * FQ4],
                )
            gblk_sb = const.tile([128, ndist * 128], BF16)
            GQ2 = (ndist * 128) // 2
            nc.sync.dma_start(out=gblk_sb[:, 0:GQ2], in_=gblk[:, 0:GQ2])
            nc.gpsimd.dma_start(out=gblk_sb[:, GQ2:], in_=gblk[:, GQ2:])
            # Big zeroed SBUF tile holding every built T-lhsT [128,128] block
            # (device-built windowed-M). Pool memset runs during stage A.
            tl_sb = const.tile([128, ndist * 128], BF16)
            nc.gpsimd.memset(tl_sb, 0.0)
            # Warm the Exp table while input DMAs run.
            warm = const.tile([1, 1], F32)
            nc.vector.memset(warm, 0.0)
            warm2 = const.tile([1, 1], F32)
            nc.scalar.activation(warm2, warm, AF.Exp)

            # -------- stage A: moment matrix m2[j1,j0] (rows at 64:112) ----
            # Stage-B argB+exp batches interleave with stage-A exp groups
            # (they depend only on featx/gblk DMAs, not on m2), so the
            # Activation engine never idles at the A->B transition. The
            # T/mul/reduce pass runs after the m2-window builds.
            pq_ctx = tc.tile_pool(name="psB", bufs=1, space="PSUM")
            psB = pq_ctx.__enter__()
            pa_ctx = tc.tile_pool(name="psA", bufs=2, space="PSUM")
            psA = pa_ctx.__enter__()
            pm_ctx = tc.tile_pool(name="psM", bufs=1, space="PSUM")
            psM = pm_ctx.__enter__()
            m2ps = psM.tile([128, NG], F32, tag="m2", bufs=1)

            NGB = (NBLK + EXPB - 1) // EXPB
            qbs = {}

            def emit_argB(g):
                nb = min(EXPB, NBLK - g * EXPB)
                ap2 = psB.tile([128, nb * 128], F32, tag="argB")
                for i in range(nb):
                    b = g * EXPB + i
                    _, d = blocks[b]
                    nc.tensor.matmul(
                        ap2[:, i * 128 : (i + 1) * 128],
                        lhsT=gblk_sb[:, d * 128 : (d + 1) * 128],
                        rhs=featx_sb[:, b * 128 : (b + 1) * 128],
                        start=True, stop=True,
                    )
                return ap2, nb

            def emit_bexp(g):
                ap2, nb = emit_argB(g)
                qb = sbQ.tile([128, nb * 128], BF16, tag="QB", bufs=NGB)
                nc.scalar.activation(qb, ap2, AF.Exp)
                qbs[g] = qb
            chorder = sorted(range(NCHUNK), key=lambda c: ((c % 16) // 4, c))
            gsizes = [AGRP] * (NCHUNK // AGRP)
            gstart = [sum(gsizes[:i]) for i in range(len(gsizes))]

            def colpack(n):
                offs, gaps, cur = [], [], 0
                for w in [NGR] * n + [w0a] * n:
                    if cur % 512 + w > 512:
                        nxt = (cur // 512 + 1) * 512
                        gaps.append((cur, nxt - cur))
                        cur = nxt
                    offs.append(cur)
                    cur += w
                return offs, gaps, cur

            zg = const.tile([1, 512], BF16)
            nc.vector.memset(zg, 0.0)

            def emit_argT(g):
                n = gsizes[g]
                offs, gaps, ACOLS = colpack(n)
                at = psA.tile([128, ACOLS], F32, tag="argT")
                for goff, gw in gaps:
                    nc.tensor.matmul(
                        at[:, goff : goff + gw], lhsT=zg[:, 0:128],
                        rhs=zg[:, 0:gw], start=True, stop=True,
                    )
                for i in range(n):
                    ch = chorder[gstart[g] + i]
                    blk, grp, j = ch // 64, (ch // 16) % 4, ch % 16
                    bs = slice(blk * 64, (blk + 1) * 64)
                    gcol = grp * 128
                    nc.tensor.matmul(
                        at[:, offs[i] : offs[i] + NGR],
                        lhsT=featd_sb[bs, j * 128 : (j + 1) * 128],
                        rhs=ga_sb[bs, gcol : gcol + NGR],
                        start=True, stop=True,
                    )
                    o0 = o0s[ch]
                    nc.tensor.matmul(
                        at[:, offs[n + i] : offs[n + i] + w0a],
                        lhsT=featd_sb[bs, j * 128 : (j + 1) * 128],
                        rhs=ga_sb[bs, gcol + 64 + o0 : gcol + 64 + o0 + w0a],
                        start=True, stop=True,
                    )
                return at, offs, n

            zz = const.tile([1, NG], BF16)
            nc.vector.memset(zz, 0.0)
            nc.tensor.matmul(
                m2ps[NG : 2 * NG, :], lhsT=zz, rhs=zz, start=True, stop=False,
                skip_group_check=True,
            )
            NGA = len(gsizes)
            ats = {0: emit_argT(0)}
            for g in range(NGA):
                at, offs, n = ats.pop(g)
                pat = sbA.tile([128, at.shape[1]], BF16, tag="PAT")
                nc.scalar.activation(pat, at, AF.Exp)
                if g + 1 < NGA:
                    ats[g + 1] = emit_argT(g + 1)
                if g < NGB:
                    emit_bexp(g)
                for i in range(n):
                    ch = chorder[gstart[g] + i]
                    nc.tensor.matmul(
                        m2ps[NG : NG + NGR, o0s[ch] : o0s[ch] + w0a],
                        lhsT=pat[:, offs[i] : offs[i] + NGR],
                        rhs=pat[:, offs[n + i] : offs[n + i] + w0a],
                        start=False,
                        stop=(gstart[g] + i == NCHUNK - 1),
                        skip_group_check=True,
                    )
            m2bf = const.tile([128, NG], BF16)
            nc.scalar.mul(m2bf[NG : NG + NGR, :], m2ps[NG : NG + NGR, :], 1.0)
            for g in range(NGA, NGB):
                emit_bexp(g)
            pm_ctx.__exit__(None, None, None)
            pa_ctx.__exit__(None, None, None)

            # -------- T-lhsT builds: windowed m2 blocks, band-placed -------
            # For distinct d (mode 4): tl[32s+16+j1, 32s+j0'] = m2[o1+j1, o0+j0']
            # (mode 2): tl[64s+16+j1, 64s+j0'] = m2[j1, o0+j0']
            # Build matmul per (d, s): lhsT = selM slice (banded diagonal:
            # selm[64+j1, 112 + j1 - (base+16) + o1shift]), rhs = m2bf col
            # window -> PSUM [128, W] with zeros outside the band; one batched
            # DVE copy scatters col-groups into tl_sb.
            pb_ctx = tc.tile_pool(name="psBLD", bufs=2, space="PSUM")
            psBLD = pb_ctx.__enter__()
            d4 = [d for d, (m, _, _, _) in enumerate(dmap) if m == 4]
            d2 = [d for d, (m, _, _, _) in enumerate(dmap) if m == 2]
            assert d4 == list(range(len(d4)))
            assert d2 == list(range(len(d4), len(d4) + len(d2)))

            def emit_builds(ds, nsub, bstride):
                # one PSUM tile holding nsub*W cols per distinct; ds must be
                # a consecutive id range so one strided copy scatters all.
                if not ds:
                    return
                per = nsub * W
                CH = max(1, 512 // per)  # distincts per PSUM tile (1 bank)
                for c0 in range(0, len(ds), CH):
                    dd = ds[c0 : c0 + CH]
                    nd = len(dd)
                    pb = psBLD.tile([128, nd * per], F32, tag="bld")
                    for i, d in enumerate(dd):
                        m, o0, o1, pidx = dmap[d]
                        for s in range(nsub):
                            off = pidx * 224 + (96 - bstride * s)
                            nc.tensor.matmul(
                                pb[:, i * per + s * W : i * per + (s + 1) * W],
                                lhsT=selm_sb[64:112, off : off + 128],
                                rhs=m2bf[64 : 64 + NGR, o0 : o0 + W],
                                start=True, stop=True,
                            )
                    # one strided scatter copy: src [p][d][s][w] contiguous,
                    # dst tl cols d*128 + s*bstride + w
                    src = pb.rearrange("p (d s w) -> p d s w", s=nsub, w=W)
                    dst = (
                        tl_sb[:, dd[0] * 128 : (dd[-1] + 1) * 128]
                        .rearrange("p (d s r) -> p d s r", d=nd, s=nsub)[
                            :, :, :, 0:W
                        ]
                    )
                    nc.vector.tensor_copy(dst, src)

            emit_builds(d4, 4, 32)
            emit_builds(d2, 2, 64)
            pb_ctx.__exit__(None, None, None)

            # -------- stage B pass 2: T-matmul / R-mul / reduce ------------
            pt_ctx = tc.tile_pool(name="psT", bufs=2, space="PSUM")
            psT = pt_ctx.__enter__()
            po_ctx = tc.tile_pool(name="psO", bufs=1, space="PSUM")
            psO = po_ctx.__enter__()
            ocol = psO.tile([128, 128], F32, tag="oc", bufs=1)

            cum = [0]
            for m, _ in blocks:
                cum.append(cum[-1] + m)
            assert cum[-1] == 128

            out2 = out.rearrange("(p q) -> p q", p=128)
            for g in range(NGB):
                nb = min(EXPB, NBLK - g * EXPB)
                qb = qbs.pop(g)
                # T-matmuls for the batch into one psT tile, then one R-mul
                tp = psT.tile([128, nb * 128], F32, tag="T")
                n1 = 0  # count of mode-1 blocks in batch (must be trailing)
                for i in range(nb):
                    b = g * EXPB + i
                    m, d = blocks[b]
                    sl = slice(i * 128, (i + 1) * 128)
                    if m == 1:
                        nc.tensor.matmul(
                            tp[0:NG, sl],
                            lhsT=m2bf[NG : NG + NGR, 0:NG],
                            rhs=qb[NG : NG + NGR, sl],
                            start=True, stop=True, skip_group_check=True,
                        )
                        n1 += 1
                    else:
                        assert n1 == 0, "mode-1 blocks must be trailing"
                        nc.tensor.matmul(
                            tp[:, sl],
                            lhsT=tl_sb[:, d * 128 : (d + 1) * 128],
                            rhs=qb[:, sl],
                            start=True, stop=True, skip_group_check=True,
                        )
                rb = sbR.tile([128, nb * 128], BF16, tag="R")
                nfull = nb - n1
                if nfull:
                    nc.vector.tensor_mul(
                        rb[:, 0 : nfull * 128], qb[:, 0 : nfull * 128],
                        tp[:, 0 : nfull * 128],
                    )
                if n1:
                    nc.vector.tensor_mul(
                        rb[0:NG, nfull * 128 : nb * 128],
                        qb[0:NG, nfull * 128 : nb * 128],
                        tp[0:NG, nfull * 128 : nb * 128],
                    )
                for i in range(nb):
                    b = g * EXPB + i
                    m, d = blocks[b]
                    sl = slice(i * 128, (i + 1) * 128)
                    if m == 4:
                        rr, oc = rb[:, sl], onesm_sb[:, 0:4]
                    elif m == 2:
                        rr, oc = rb[:, sl], onesm_sb[:, 4:6]
                    else:
                        rr, oc = rb[0:NG, sl], onesm_sb[0:NG, 6:7]
                    nc.tensor.matmul(
                        ocol[:, cum[b] : cum[b + 1]], lhsT=rr, rhs=oc,
                        start=True, stop=True, skip_group_check=True,
                    )
            osb1 = const.tile([128, 128], F32, tag="osb")
            nc.scalar.activation(osb1, ocol, AF.Copy)
            nc.sync.dma_start(out=out2, in_=osb1)
            po_ctx.__exit__(None, None, None)
            pt_ctx.__exit__(None, None, None)
            pq_ctx.__exit__(None, None, None)
    return nc


_CACHE = {}
LAST_RESULTS = None
LAST_KEY = None


def _get_nc():
    key = LAST_KEY
    if key not in _CACHE:
        nc = bacc.Bacc("TRN2", target_bir_lowering=False, debug=False)
        _build_core_program(nc, key)
        nc.compile()
        _CACHE[key] = nc
    return _CACHE[key]


def _split(v):
    hi = v.astype(BF)
    lo = (v - hi.astype(np.float64)).astype(BF)
    return hi, lo


def _feat6(v):
    v = v.astype(np.float64)
    v_hi, v_lo = _split(v)
    s = A * v * v
    s_hi, s_lo = _split(s)
    return [v_hi, v_hi, v_lo, v_lo, s_hi, s_lo]


def _host_prep_a(x, data, weights):
    """Stage-A host prep (unchanged from v2): featd rows, ga coefficients,
    chunk windows; returns grid params too."""
    lo = float(min(x.min(), data.min())) - PADG
    hi = float(max(x.max(), data.max())) + PADG
    H = max(HMIN, (hi - lo) / (NGR - 1))
    assert H <= 1.1, f"range {hi - lo} too wide for {NGR} nodes"
    FQ = float(H * np.sqrt(2.0 * A / np.pi))
    u = lo + np.arange(NGR) * H

    c_hi, c_lo = _split(2.0 * A * u)
    b_hi, b_lo = _split(-A * u * u)

    gb6 = np.zeros((6, NGR), dtype=np.float64)
    gb6[0] = c_hi.astype(np.float64)
    gb6[1] = c_lo.astype(np.float64)
    gb6[2] = c_hi.astype(np.float64)
    gb6[3] = c_lo.astype(np.float64)
    gb6[4] = -1.0
    gb6[5] = -1.0
    ga = np.zeros((KD, 2 * NG), dtype=np.float64)
    ga[:6, NG : NG + NGR] = gb6
    ga[6:12, :NGR] = gb6
    ga[12, NG : NG + NGR] = 1.0
    ga[13, NG : NG + NGR] = 1.0
    bh = np.zeros(NG)
    bh[:NGR] = b_hi.astype(np.float64)
    bl = np.zeros(NG)
    bl[:NGR] = b_lo.astype(np.float64)
    ga[14, :] = np.tile(bh, 2)
    ga[15, :] = np.tile(bl, 2)

    d64 = data.astype(np.float64)
    dperm = np.argsort(d64[:, 0], kind="stable")
    dsort = d64[dperm]
    wsort = weights.astype(np.float64)[dperm]
    d0c = dsort[:, 0].reshape(NCHUNK, 128)
    lo_node = np.floor((d0c.min(axis=1) - RPADA - lo) / H).astype(int)
    hi_node = np.ceil((d0c.max(axis=1) + RPADA - lo) / H).astype(int)
    if (hi_node - lo_node + 1).max() <= W0A:
        w0a = W0A
        o0s = np.clip(lo_node, 0, NGR - W0A)
    else:
        w0a = NGR
        o0s = np.zeros(NCHUNK, dtype=int)

    lnw = np.log(np.maximum(wsort, 1e-300))
    lnw = np.maximum(lnw + 2.0 * np.log(FQ), -60.0)
    lnw_hi, lnw_lo = _split(lnw)
    ones = np.ones(NPTS, dtype=BF)
    featd = np.stack(
        _feat6(dsort[:, 0]) + _feat6(dsort[:, 1]) + [lnw_hi, lnw_lo, ones, ones]
    )
    return featd.astype(BF), ga, w0a, o0s, lo, H, u, (c_hi, c_lo, b_hi, b_lo)


def _widen(rows, n):
    k = rows.shape[0]
    w = np.zeros((8, 16, n // 8), dtype=BF)
    w[:, :k] = rows.reshape(k, 8, n // 8).transpose(1, 0, 2)
    return w.reshape(128, n // 8)


def _grep(g):
    m = g.shape[1]
    r = np.zeros((8, 16, 4, m), dtype=np.float64)
    for grp in range(4):
        for blk in range(2):
            r[blk * 4 + grp, : g.shape[0], grp] = g
    return r.reshape(128, 4 * m).astype(BF)


def _make_schedule(xs, H, glo):
    """Shared-across-cores tile schedule. xs: [NCORES, NSH, 2] float64.
    Returns blocks [(mode, o0, o1, rank_lists per core...)]. Tiles are
    formed on aligned sorted-rank ranges; windows cover the union box."""
    BUD = 15 * H - 2 * REACH - QUANT * H
    ord0 = np.argsort(xs[:, :, 0], axis=1, kind="stable")
    x0s = np.take_along_axis(xs[:, :, 0], ord0, axis=1)  # sorted x0 per core

    def span0(i, j):
        return float((x0s[:, j - 1] - x0s[:, i]).max())

    bands = []
    i = 0
    while i < NSH:
        j = i + 128
        while j + 128 <= NSH and (j - i) < 4096 and span0(i, j + 128) <= BUD:
            j += 128
        bands.append((i, j))
        i = j

    tiles = []  # (mode, per-core index arrays, o0, o1)
    for (i, j) in bands:
        # per-core: locs of this band sorted by x1
        bidx = [ord0[c, i:j] for c in range(NCORES)]
        b1 = [bidx[c][np.argsort(xs[c, bidx[c], 1], kind="stable")]
              for c in range(NCORES)]
        bs0 = span0(i, j)
        k = 0
        n = j - i
        while k < n:
            rem = n - k
            placed = False
            if rem >= 512 and bs0 <= BUD:
                idxs = [b1[c][k : k + 512] for c in range(NCORES)]
                sp1 = max(
                    float(xs[c, idxs[c], 1].max() - xs[c, idxs[c], 1].min())
                    for c in range(NCORES)
                )
                sp0 = max(
                    float(xs[c, idxs[c], 0].max() - xs[c, idxs[c], 0].min())
                    for c in range(NCORES)
                )
                if sp1 <= BUD and sp0 <= BUD:
                    tiles.append((4, idxs))
                    k += 512
                    placed = True
            if not placed and rem >= 256:
                idxs = [b1[c][k : k + 256] for c in range(NCORES)]
                sp0 = max(
                    float(xs[c, idxs[c], 0].max() - xs[c, idxs[c], 0].min())
                    for c in range(NCORES)
                )
                if sp0 <= BUD:
                    tiles.append((2, idxs))
                    k += 256
                    placed = True
            if not placed:
                tiles.append((1, [b1[c][k : k + 128] for c in range(NCORES)]))
                k += 128

    def win_off(vals_min, vals_max):
        o = int(np.floor((vals_min - REACH - glo) / H))
        o = (o // QUANT) * QUANT
        o = max(0, min(NGR - W, o))
        assert glo + (o + W - 1) * H >= vals_max + REACH - 1e-9
        return o

    sched = []
    for mode, idxs in tiles:
        o0 = o1 = 0
        if mode in (4, 2):
            o0 = win_off(
                min(float(xs[c, idxs[c], 0].min()) for c in range(NCORES)),
                max(float(xs[c, idxs[c], 0].max()) for c in range(NCORES)),
            )
        if mode == 4:
            o1 = win_off(
                min(float(xs[c, idxs[c], 1].min()) for c in range(NCORES)),
                max(float(xs[c, idxs[c], 1].max()) for c in range(NCORES)),
            )
        sched.append((mode, o0, o1, idxs))
    # order: mode-4 blocks, then mode-2, then mode-1 (trailing for mul split)
    sched.sort(key=lambda t: -t[0])
    return sched


def _coef_tile(mode, o0, o1, cb):
    """Host block-diagonal argB lhsT [128,128] bf16 for one distinct."""
    c_hi, c_lo, b_hi, b_lo = cb
    t = np.zeros((128, 128), dtype=np.float64)

    def put(colbase, rowbase, nodes, dim):
        # feature rows within a sub-block: dim0 at rows 0..5, dim1 6..11,
        # ones 12..13
        fr = rowbase + (0 if dim == 0 else 6)
        for ci, j in enumerate(nodes):
            col = colbase + ci
            t[fr + 0, col] = c_hi[j]
            t[fr + 1, col] = c_lo[j]
            t[fr + 2, col] = c_hi[j]
            t[fr + 3, col] = c_lo[j]
            t[fr + 4, col] = -1.0
            t[fr + 5, col] = -1.0
            t[rowbase + 12, col] = b_hi[j]
            t[rowbase + 13, col] = b_lo[j]

    def pad(colbase, rowbase, ncols):
        t[rowbase + 12, colbase : colbase + ncols] = -4900.0

    if mode == 4:
        for s in range(4):
            put(32 * s, 32 * s, range(o0, o0 + W), 0)
            put(32 * s + W, 32 * s, range(o1, o1 + W), 1)
    elif mode == 2:
        for s in range(2):
            put(64 * s, 64 * s, range(o0, o0 + W), 0)
            put(64 * s + W, 64 * s, range(NGR), 1)
    else:
        put(0, 0, range(NGR), 0)
        pad(NGR, 0, NG - NGR)
        put(NG, 0, range(NGR), 1)
        pad(NG + NGR, 0, NG - NGR)
    return t.astype(BF)


def make_in_maps(x, data, weights):
    featd, ga, w0a, o0s, glo, H, u, cb = _host_prep_a(x, data, weights)
    xs = x.reshape(NCORES, NSH, D).astype(np.float64)
    sched = _make_schedule(xs, H, glo)

    # distinct map (sched is mode-desc sorted => d4 ids, then d2, then d1)
    dmap0, did = [], {}
    blocks = []
    for mode, o0, o1, idxs in sched:
        k = (mode, o0, o1)
        if k not in did:
            did[k] = len(dmap0)
            dmap0.append(k)
        blocks.append((mode, did[k]))
    ndist = len(dmap0)

    # selector panels: one per distinct mode-4 o1 value, plus a full-diagonal
    # panel (-1) for mode-2
    pvals = sorted({o1 for (m, o0, o1) in dmap0 if m == 4})
    if any(m == 2 for (m, _, _) in dmap0):
        pvals = pvals + [-1]
    pidx_of = {v: i for i, v in enumerate(pvals)}
    dmap = []
    for m, o0, o1 in dmap0:
        pidx = pidx_of[o1] if m == 4 else (pidx_of[-1] if m == 2 else 0)
        dmap.append((m, o0, o1, pidx))

    global LAST_KEY
    LAST_KEY = (
        int(w0a), tuple(int(v) for v in o0s), tuple(blocks), ndist,
        tuple(dmap), tuple(pvals),
    )

    # featx packing + output gather per core
    NBLK = len(blocks)
    featx = np.zeros((NCORES, 128, NBLK * 128), dtype=np.float64)
    gather = np.zeros((NCORES, NSH), dtype=np.int64)  # loc <- dram index
    cum = 0
    x64 = xs
    for b, (mode, o0, o1, idxs) in enumerate(sched):
        m = mode
        sub = 128 // m
        for c in range(NCORES):
            idx = idxs[c]
            xt = x64[c, idx]  # [nl, 2]
            f0 = np.stack(_feat6(xt[:, 0])).astype(np.float64)  # [6, nl]
            f1 = np.stack(_feat6(xt[:, 1])).astype(np.float64)
            for s in range(m):
                li = np.arange(s * 128, (s + 1) * 128)
                rows = sub * s
                featx[c, rows : rows + 6, b * 128 : (b + 1) * 128] = f0[:, li]
                featx[c, rows + 6 : rows + 12, b * 128 : (b + 1) * 128] = f1[:, li]
                featx[c, rows + 12 : rows + 14, b * 128 : (b + 1) * 128] = 1.0
                # dram index = p*128 + (cum + s); loc = idx[s*128 + p]
                gather[c, idx[li]] = np.arange(128) * 128 + (cum + s)
        cum += m
    assert cum == 128

    gblk = np.zeros((128, ndist * 128), dtype=BF)
    for d, (mode, o0, o1, pidx) in enumerate(dmap):
        gblk[:, d * 128 : (d + 1) * 128] = _coef_tile(mode, o0, o1, cb)

    # selector panels [48, 224] each: panel for o1 value v has
    # panel[j1, 112 + j1 - v] = 1 for j1 in [v, v+W) (v=-1: all 48 rows,
    # diagonal at 112 + j1). Sliced at off = 96 - base so lhsT local col
    # k = base+16+j1m hits j1 = v+j1m.
    selm = np.zeros((48, len(pvals) * 224), dtype=BF)
    for i, v in enumerate(pvals):
        if v < 0:
            for j1 in range(NGR):
                selm[j1, i * 224 + 112 + j1] = 1.0
        else:
            for j1m in range(W):
                selm[v + j1m, i * 224 + 112 + j1m] = 1.0
    onesm = np.zeros((128, 8), dtype=BF)
    for s in range(4):
        onesm[32 * s : 32 * (s + 1), s] = 1.0
    for s in range(2):
        onesm[64 * s : 64 * (s + 1), 4 + s] = 1.0
    onesm[0:NG, 6] = 1.0

    featd_w = _widen(featd, NPTS)
    ga_w = _grep(ga)
    in_maps = []
    for c in range(NCORES):
        in_maps.append({
            "featd": featd_w,
            "ga": ga_w,
            "featx": featx[c].astype(BF),
            "gblk": gblk,
            "selm": selm,
            "onesm": onesm,
        })
    return in_maps, gather


def kernel(x, data, weights):
    global LAST_RESULTS
    x = np.ascontiguousarray(x, dtype=np.float32)
    data = np.ascontiguousarray(data, dtype=np.float32)
    weights = np.ascontiguousarray(weights, dtype=np.float32)
    assert x.shape == (B, L, D) and data.shape == (NPTS, D)

    in_maps, gather = make_in_maps(x, data, weights)
    nc = _get_nc()
    try:
        res = bass_utils.run_bass_kernel_spmd(
            nc, in_maps, core_ids=list(range(NCORES)),
            trace=bool(os.environ.get("BASS_TRACE")),
        )
    except ModuleNotFoundError:
        os.environ["BASS_NEVER_TRACE"] = "1"
        res = bass_utils.run_bass_kernel_spmd(
            nc, in_maps, core_ids=list(range(NCORES)), trace=False,
        )
    LAST_RESULTS = res
    outs = []
    for c in range(NCORES):
        buf = res.results[c]["out"]
        outs.append(buf[gather[c]])
    return np.concatenate(outs).reshape(B, L).astype(np.float32)


# revision 20
# speedup vs baseline: 1.0553x; 1.0553x over previous
"""Weighted 2D Gaussian KDE on 8 Trainium2 NeuronCores (Bass/Tile), v3.

out[b,l] = sum_n w[n] * exp(-||x[b,l] - data[n]||^2 / sigma),  sigma = 3.

Grid-quadrature factorization (v2): with a uniform grid u_j (spacing h,
a = 2/sigma, F = h*sqrt(2a/pi)),
    out[c] = q0(x_c)^T (F^2 P1 diag(w) P0^T) q1(x_c),
    P_d[j,n] = exp(-a(u_j - d_nd)^2),  q_d[j,c] = exp(-a(u_j - x_cd)^2).

v3 redesign: stage B packs MULTIPLE locations per exp column by windowing
the grid support of each location (gaussian decay => ~10 nodes matter per
dim). Locations are 2D-sorted into tiles sharing a window pair (o0, o1):
  mode 4: col = 4 locs x [win0(16) | win1(16)] stacked in 32-row bands
  mode 2: col = 2 locs x [win0(16) | dim1 full(48)]
  mode 1: col = 1 loc  x [dim0(48) | pad | dim1(48) | pad]  (x tails)
This cuts stage-B exp/mul/matmul free-dim cost ~3.3x (the critical-path
Activation engine runs ~1 col per 2-4 locations instead of 1 per loc).
The tile schedule is SHARED across cores (SPMD: one program) by forming
tiles on aligned sorted-rank ranges and windowing the union box over all
8 cores. argB lhsT tiles are host-built block-diagonal coefficient
matrices (one per distinct (mode,o0,o1)); the windowed-M T-matmul lhsT
tiles are built on device from m2 via banded-diagonal selector matmuls
(selM const sliced per (o1,sub)) + one batched PSUM->SBUF copy.

Sharding: locations (B*L = 131072) split contiguously across 8 cores
(16384 each); data/weights replicated; moment matrix computed
redundantly on every core (collectives cost >=15us fixed).
"""

import os
import numpy as np
import ml_dtypes

import concourse.bass as bass
import concourse.tile as tile
from concourse import bacc
from concourse import mybir
from concourse import bass_utils

BF = ml_dtypes.bfloat16

# ---- problem constants (hardcoded per spec) ----
B, L, D = 2, 65536, 2
NPTS = 16384
NCORES = 8
NLOC = B * L              # 131072 locations
NSH = NLOC // NCORES      # 16384 per core
SIGMA = 3.0
A = 2.0 / SIGMA
PADG = 3.0                # grid extension beyond data/location range
NG = 64                   # partition stride per dim for stage A layout
NGR = 48                  # real grid node count; spacing adapts to range
HMIN = 0.75
W = 16                    # stage-B window nodes per dim (modes 4/2)
REACH = 3.0               # min gaussian reach beyond a tile's box
QUANT = 2                 # window offset quantization (node units)

KD = 16                   # featd rows
NCHUNK = NPTS // 128      # 128 data chunks
AGRP = 16                 # stage-A chunks per exp batch
W0A = 20                  # stage-A dim0 grid window (nodes) per chunk
RPADA = 4.0               # stage-A window reach beyond a chunk's d0 range
EXPB = 4                  # stage-B blocks per exp batch (128 cols each)

F32 = mybir.dt.float32
BF16 = mybir.dt.bfloat16
AF = mybir.ActivationFunctionType


def _build_core_program(nc: bass.Bass, key):
    w0a, o0s, blocks, ndist, dmap, panels = key
    # blocks: tuple of (mode, dist_id); dmap: (mode, o0, o1, panel_idx) per
    # dist; panels: tuple of o1 values (-1 = full 48-row diagonal, mode-2)
    NBLK = len(blocks)
    TCOLS = NBLK * 128
    NPAN = len(panels)

    featd = nc.dram_tensor("featd", [128, 2048], BF16, kind="ExternalInput").ap()
    ga = nc.dram_tensor("ga", [128, 8 * NG], BF16, kind="ExternalInput").ap()
    featx = nc.dram_tensor("featx", [128, TCOLS], BF16, kind="ExternalInput").ap()
    gblk = nc.dram_tensor("gblk", [128, ndist * 128], BF16, kind="ExternalInput").ap()
    selm = nc.dram_tensor("selm", [48, NPAN * 224], BF16, kind="ExternalInput").ap()
    onesm = nc.dram_tensor("onesm", [128, 8], BF16, kind="ExternalInput").ap()
    out = nc.dram_tensor("out", [NSH], F32, kind="ExternalOutput").ap()

    with tile.TileContext(nc) as tc:
        with (
            tc.tile_pool(name="const", bufs=1) as const,
            tc.tile_pool(name="sbA", bufs=2) as sbA,
            tc.tile_pool(name="sbQ", bufs=6) as sbQ,
            tc.tile_pool(name="sbR", bufs=4) as sbR,
        ):
            # featd halves + ga first on separate DGE queues so stage A can
            # start ASAP. Nothing on the Activation queue (sequencer must be
            # free for the first exp).
            featd_sb = const.tile([128, 2048], BF16)
            Q4 = 2048 // 4
            nc.sync.dma_start(out=featd_sb[:, 0:Q4], in_=featd[:, 0:Q4])
            ga_sb = const.tile([128, 8 * NG], BF16)
            nc.sync.dma_start(out=ga_sb, in_=ga)
            for qi, qeng in [(1, nc.gpsimd), (2, nc.sync), (3, nc.gpsimd)]:
                qeng.dma_start(
                    out=featd_sb[:, qi * Q4 : (qi + 1) * Q4],
                    in_=featd[:, qi * Q4 : (qi + 1) * Q4],
                )
            selm_sb = const.tile([128, NPAN * 224], BF16)
            nc.sync.dma_start(out=selm_sb[64:112, :], in_=selm)
            onesm_sb = const.tile([128, 8], BF16)
            nc.sync.dma_start(out=onesm_sb, in_=onesm)
            featx_sb = const.tile([128, TCOLS], BF16)
            FQ4 = TCOLS // 4
            for qi, qeng in [(0, nc.sync), (1, nc.gpsimd), (2, nc.sync),
                             (3, nc.gpsimd)]:
                qeng.dma_start(
                    out=featx_sb[:, qi * FQ4 : (qi + 1) * FQ4],
                    in_=featx[:, qi * FQ4 : (qi + 1) * FQ4],
                )
            gblk_sb = const.tile([128, ndist * 128], BF16)
            GQ2 = (ndist * 128) // 2
            nc.sync.dma_start(out=gblk_sb[:, 0:GQ2], in_=gblk[:, 0:GQ2])
            nc.gpsimd.dma_start(out=gblk_sb[:, GQ2:], in_=gblk[:, GQ2:])
            # Big zeroed SBUF tile holding every built T-lhsT [128,128] block
            # (device-built windowed-M). Pool memset runs during stage A.
            tl_sb = const.tile([128, ndist * 128], BF16)
            nc.gpsimd.memset(tl_sb, 0.0)
            # Warm the Exp table while input DMAs run.
            warm = const.tile([1, 1], F32)
            nc.vector.memset(warm, 0.0)
            warm2 = const.tile([1, 1], F32)
            nc.scalar.activation(warm2, warm, AF.Exp)

            # -------- stage A: moment matrix m2[j1,j0] (rows at 64:112) ----
            # Stage-B argB+exp batches interleave with stage-A exp groups
            # (they depend only on featx/gblk DMAs, not on m2), so the
            # Activation engine never idles at the A->B transition. The
            # T/mul/reduce pass runs after the m2-window builds.
            pq_ctx = tc.tile_pool(name="psB", bufs=1, space="PSUM")
            psB = pq_ctx.__enter__()
            pa_ctx = tc.tile_pool(name="psA", bufs=2, space="PSUM")
            psA = pa_ctx.__enter__()
            pm_ctx = tc.tile_pool(name="psM", bufs=1, space="PSUM")
            psM = pm_ctx.__enter__()
            m2ps = psM.tile([128, NG], F32, tag="m2", bufs=1)

            NGB = (NBLK + EXPB - 1) // EXPB
            qbs = {}

            def emit_bexp(g):
                nb = min(EXPB, NBLK - g * EXPB)
                ap2 = psB.tile([128, nb * 128], F32, tag="argB")
                for i in range(nb):
                    b = g * EXPB + i
                    _, d = blocks[b]
                    nc.tensor.matmul(
                        ap2[:, i * 128 : (i + 1) * 128],
                        lhsT=gblk_sb[:, d * 128 : (d + 1) * 128],
                        rhs=featx_sb[:, b * 128 : (b + 1) * 128],
                        start=True, stop=True,
                    )
                qb = sbQ.tile([128, nb * 128], BF16, tag="QB", bufs=NGB)
                nc.scalar.activation(qb, ap2, AF.Exp)
                qbs[g] = qb
            chorder = sorted(range(NCHUNK), key=lambda c: ((c % 16) // 4, c))
            gsizes = [AGRP] * (NCHUNK // AGRP)
            if NCHUNK % AGRP:
                gsizes.append(NCHUNK % AGRP)
            gstart = [sum(gsizes[:i]) for i in range(len(gsizes))]

            def colpack(n):
                offs, gaps, cur = [], [], 0
                for w in [NGR] * n + [w0a] * n:
                    if cur % 512 + w > 512:
                        nxt = (cur // 512 + 1) * 512
                        gaps.append((cur, nxt - cur))
                        cur = nxt
                    offs.append(cur)
                    cur += w
                return offs, gaps, cur

            zg = const.tile([1, 512], BF16)
            nc.vector.memset(zg, 0.0)

            def emit_argT(g):
                n = gsizes[g]
                offs, gaps, ACOLS = colpack(n)
                at = psA.tile([128, ACOLS], F32, tag="argT")
                for goff, gw in gaps:
                    nc.tensor.matmul(
                        at[:, goff : goff + gw], lhsT=zg[:, 0:128],
                        rhs=zg[:, 0:gw], start=True, stop=True,
                    )
                for i in range(n):
                    ch = chorder[gstart[g] + i]
                    blk, grp, j = ch // 64, (ch // 16) % 4, ch % 16
                    bs = slice(blk * 64, (blk + 1) * 64)
                    gcol = grp * 128
                    nc.tensor.matmul(
                        at[:, offs[i] : offs[i] + NGR],
                        lhsT=featd_sb[bs, j * 128 : (j + 1) * 128],
                        rhs=ga_sb[bs, gcol : gcol + NGR],
                        start=True, stop=True,
                    )
                    o0 = o0s[ch]
                    nc.tensor.matmul(
                        at[:, offs[n + i] : offs[n + i] + w0a],
                        lhsT=featd_sb[bs, j * 128 : (j + 1) * 128],
                        rhs=ga_sb[bs, gcol + 64 + o0 : gcol + 64 + o0 + w0a],
                        start=True, stop=True,
                    )
                return at, offs, n

            zz = const.tile([1, NG], BF16)
            nc.vector.memset(zz, 0.0)
            nc.tensor.matmul(
                m2ps[NG : 2 * NG, :], lhsT=zz, rhs=zz, start=True, stop=False,
                skip_group_check=True,
            )
            NGA = len(gsizes)
            ats = {0: emit_argT(0)}
            for g in range(NGA):
                at, offs, n = ats.pop(g)
                pat = sbA.tile([128, at.shape[1]], BF16, tag="PAT")
                nc.scalar.activation(pat, at, AF.Exp)
                if g + 1 < NGA:
                    ats[g + 1] = emit_argT(g + 1)
                for i in range(n):
                    ch = chorder[gstart[g] + i]
                    nc.tensor.matmul(
                        m2ps[NG : NG + NGR, o0s[ch] : o0s[ch] + w0a],
                        lhsT=pat[:, offs[i] : offs[i] + NGR],
                        rhs=pat[:, offs[n + i] : offs[n + i] + w0a],
                        start=False,
                        stop=(gstart[g] + i == NCHUNK - 1),
                        skip_group_check=True,
                    )
            m2bf = const.tile([128, NG], BF16)
            nc.scalar.mul(m2bf[NG : NG + NGR, :], m2ps[NG : NG + NGR, :], 1.0)
            # first argB+exp issued BEFORE the builds so the Activation
            # engine rolls straight from stage-A exps into stage-B exps
            # while the PE does the m2-window builds.
            emit_bexp(0)
            pm_ctx.__exit__(None, None, None)
            pa_ctx.__exit__(None, None, None)

            # -------- T-lhsT builds: windowed m2 blocks, band-placed -------
            # For distinct d (mode 4): tl[32s+16+j1, 32s+j0'] = m2[o1+j1, o0+j0']
            # (mode 2): tl[64s+16+j1, 64s+j0'] = m2[j1, o0+j0']
            # Build matmul per (d, s): lhsT = selM slice (banded diagonal:
            # selm[64+j1, 112 + j1 - (base+16) + o1shift]), rhs = m2bf col
            # window -> PSUM [128, W] with zeros outside the band; one batched
            # DVE copy scatters col-groups into tl_sb.
            pb_ctx = tc.tile_pool(name="psBLD", bufs=2, space="PSUM")
            psBLD = pb_ctx.__enter__()
            d4 = [d for d, (m, _, _, _) in enumerate(dmap) if m == 4]
            d2 = [d for d, (m, _, _, _) in enumerate(dmap) if m == 2]
            assert d4 == list(range(len(d4)))
            assert d2 == list(range(len(d4), len(d4) + len(d2)))

            def emit_builds(ds, nsub, bstride):
                # one PSUM tile holding nsub*W cols per distinct; ds must be
                # a consecutive id range so one strided copy scatters all.
                if not ds:
                    return
                per = nsub * W
                CH = max(1, 512 // per)  # distincts per PSUM tile (1 bank)
                for c0 in range(0, len(ds), CH):
                    dd = ds[c0 : c0 + CH]
                    nd = len(dd)
                    pb = psBLD.tile([128, nd * per], F32, tag="bld")
                    for i, d in enumerate(dd):
                        m, o0, o1, pidx = dmap[d]
                        for s in range(nsub):
                            off = pidx * 224 + (96 - bstride * s)
                            nc.tensor.matmul(
                                pb[:, i * per + s * W : i * per + (s + 1) * W],
                                lhsT=selm_sb[64:112, off : off + 128],
                                rhs=m2bf[64 : 64 + NGR, o0 : o0 + W],
                                start=True, stop=True,
                            )
                    # one strided scatter copy: src [p][d][s][w] contiguous,
                    # dst tl cols d*128 + s*bstride + w
                    src = pb.rearrange("p (d s w) -> p d s w", s=nsub, w=W)
                    dst = (
                        tl_sb[:, dd[0] * 128 : (dd[-1] + 1) * 128]
                        .rearrange("p (d s r) -> p d s r", d=nd, s=nsub)[
                            :, :, :, 0:W
                        ]
                    )
                    nc.vector.tensor_copy(dst, src)

            emit_builds(d4, 4, 32)
            emit_builds(d2, 2, 64)
            pb_ctx.__exit__(None, None, None)

            # -------- stage B pass 2: T-matmul / R-mul / reduce ------------
            pt_ctx = tc.tile_pool(name="psT", bufs=2, space="PSUM")
            psT = pt_ctx.__enter__()
            po_ctx = tc.tile_pool(name="psO", bufs=1, space="PSUM")
            psO = po_ctx.__enter__()
            ocol = psO.tile([128, 128], F32, tag="oc", bufs=1)

            cum = [0]
            for m, _ in blocks:
                cum.append(cum[-1] + m)
            assert cum[-1] == 128

            out2 = out.rearrange("(p q) -> p q", p=128)
            for g in range(NGB):
                if g + 1 < NGB:
                    emit_bexp(g + 1)
                nb = min(EXPB, NBLK - g * EXPB)
                qb = qbs.pop(g)
                # T-matmuls for the batch into one psT tile, then one R-mul
                tp = psT.tile([128, nb * 128], F32, tag="T")
                n1 = 0  # count of mode-1 blocks in batch (must be trailing)
                for i in range(nb):
                    b = g * EXPB + i
                    m, d = blocks[b]
                    sl = slice(i * 128, (i + 1) * 128)
                    if m == 1:
                        nc.tensor.matmul(
                            tp[0:NG, sl],
                            lhsT=m2bf[NG : NG + NGR, 0:NG],
                            rhs=qb[NG : NG + NGR, sl],
                            start=True, stop=True, skip_group_check=True,
                        )
                        n1 += 1
                    else:
                        assert n1 == 0, "mode-1 blocks must be trailing"
                        nc.tensor.matmul(
                            tp[:, sl],
                            lhsT=tl_sb[:, d * 128 : (d + 1) * 128],
                            rhs=qb[:, sl],
                            start=True, stop=True, skip_group_check=True,
                        )
                rb = sbR.tile([128, nb * 128], BF16, tag="R")
                nfull = nb - n1
                if nfull:
                    nc.vector.tensor_mul(
                        rb[:, 0 : nfull * 128], qb[:, 0 : nfull * 128],
                        tp[:, 0 : nfull * 128],
                    )
                if n1:
                    nc.vector.tensor_mul(
                        rb[0:NG, nfull * 128 : nb * 128],
                        qb[0:NG, nfull * 128 : nb * 128],
                        tp[0:NG, nfull * 128 : nb * 128],
                    )
                for i in range(nb):
                    b = g * EXPB + i
                    m, d = blocks[b]
                    sl = slice(i * 128, (i + 1) * 128)
                    if m == 4:
                        rr, oc = rb[:, sl], onesm_sb[:, 0:4]
                    elif m == 2:
                        rr, oc = rb[:, sl], onesm_sb[:, 4:6]
                    else:
                        rr, oc = rb[0:NG, sl], onesm_sb[0:NG, 6:7]
                    nc.tensor.matmul(
                        ocol[:, cum[b] : cum[b + 1]], lhsT=rr, rhs=oc,
                        start=True, stop=True, skip_group_check=True,
                    )
            osb1 = const.tile([128, 128], F32, tag="osb")
            nc.scalar.activation(osb1, ocol, AF.Copy)
            nc.sync.dma_start(out=out2, in_=osb1)
            po_ctx.__exit__(None, None, None)
            pt_ctx.__exit__(None, None, None)
            pq_ctx.__exit__(None, None, None)
    return nc


_CACHE = {}
LAST_RESULTS = None
LAST_KEY = None


def _get_nc():
    key = LAST_KEY
    if key not in _CACHE:
        nc = bacc.Bacc("TRN2", target_bir_lowering=False, debug=False)
        _build_core_program(nc, key)
        nc.compile()
        _CACHE[key] = nc
    return _CACHE[key]


def _split(v):
    hi = v.astype(BF)
    lo = (v - hi.astype(np.float64)).astype(BF)
    return hi, lo


def _feat6(v):
    v = v.astype(np.float64)
    v_hi, v_lo = _split(v)
    s = A * v * v
    s_hi, s_lo = _split(s)
    return [v_hi, v_hi, v_lo, v_lo, s_hi, s_lo]


def _host_prep_a(x, data, weights):
    """Stage-A host prep (unchanged from v2): featd rows, ga coefficients,
    chunk windows; returns grid params too."""
    lo = float(min(x.min(), data.min())) - PADG
    hi = float(max(x.max(), data.max())) + PADG
    H = max(HMIN, (hi - lo) / (NGR - 1))
    assert H <= 1.1, f"range {hi - lo} too wide for {NGR} nodes"
    FQ = float(H * np.sqrt(2.0 * A / np.pi))
    u = lo + np.arange(NGR) * H

    c_hi, c_lo = _split(2.0 * A * u)
    b_hi, b_lo = _split(-A * u * u)

    gb6 = np.zeros((6, NGR), dtype=np.float64)
    gb6[0] = c_hi.astype(np.float64)
    gb6[1] = c_lo.astype(np.float64)
    gb6[2] = c_hi.astype(np.float64)
    gb6[3] = c_lo.astype(np.float64)
    gb6[4] = -1.0
    gb6[5] = -1.0
    ga = np.zeros((KD, 2 * NG), dtype=np.float64)
    ga[:6, NG : NG + NGR] = gb6
    ga[6:12, :NGR] = gb6
    ga[12, NG : NG + NGR] = 1.0
    ga[13, NG : NG + NGR] = 1.0
    bh = np.zeros(NG)
    bh[:NGR] = b_hi.astype(np.float64)
    bl = np.zeros(NG)
    bl[:NGR] = b_lo.astype(np.float64)
    ga[14, :] = np.tile(bh, 2)
    ga[15, :] = np.tile(bl, 2)

    d64 = data.astype(np.float64)
    dperm = np.argsort(d64[:, 0], kind="stable")
    dsort = d64[dperm]
    wsort = weights.astype(np.float64)[dperm]
    d0c = dsort[:, 0].reshape(NCHUNK, 128)
    lo_node = np.floor((d0c.min(axis=1) - RPADA - lo) / H).astype(int)
    hi_node = np.ceil((d0c.max(axis=1) + RPADA - lo) / H).astype(int)
    if (hi_node - lo_node + 1).max() <= W0A:
        w0a = W0A
        o0s = np.clip(lo_node, 0, NGR - W0A)
    else:
        w0a = NGR
        o0s = np.zeros(NCHUNK, dtype=int)

    lnw = np.log(np.maximum(wsort, 1e-300))
    lnw = np.maximum(lnw + 2.0 * np.log(FQ), -60.0)
    lnw_hi, lnw_lo = _split(lnw)
    ones = np.ones(NPTS, dtype=BF)
    featd = np.stack(
        _feat6(dsort[:, 0]) + _feat6(dsort[:, 1]) + [lnw_hi, lnw_lo, ones, ones]
    )
    return featd.astype(BF), ga, w0a, o0s, lo, H, u, (c_hi, c_lo, b_hi, b_lo)


def _widen(rows, n):
    k = rows.shape[0]
    w = np.zeros((8, 16, n // 8), dtype=BF)
    w[:, :k] = rows.reshape(k, 8, n // 8).transpose(1, 0, 2)
    return w.reshape(128, n // 8)


def _grep(g):
    m = g.shape[1]
    r = np.zeros((8, 16, 4, m), dtype=np.float64)
    for grp in range(4):
        for blk in range(2):
            r[blk * 4 + grp, : g.shape[0], grp] = g
    return r.reshape(128, 4 * m).astype(BF)


def _make_schedule(xs, H, glo):
    """Shared-across-cores tile schedule. xs: [NCORES, NSH, 2] float64.
    Returns blocks [(mode, o0, o1, rank_lists per core...)]. Tiles are
    formed on aligned sorted-rank ranges; windows cover the union box."""
    BUD = 15 * H - 2 * REACH - QUANT * H
    ord0 = np.argsort(xs[:, :, 0], axis=1, kind="stable")
    x0s = np.take_along_axis(xs[:, :, 0], ord0, axis=1)  # sorted x0 per core

    def span0(i, j):
        return float((x0s[:, j - 1] - x0s[:, i]).max())

    bands = []
    i = 0
    while i < NSH:
        j = i + 128
        while j + 128 <= NSH and (j - i) < 4096 and span0(i, j + 128) <= BUD:
            j += 128
        bands.append((i, j))
        i = j

    tiles = []  # (mode, per-core index arrays, o0, o1)
    for (i, j) in bands:
        # per-core: locs of this band sorted by x1
        bidx = [ord0[c, i:j] for c in range(NCORES)]
        b1 = [bidx[c][np.argsort(xs[c, bidx[c], 1], kind="stable")]
              for c in range(NCORES)]
        bs0 = span0(i, j)
        k = 0
        n = j - i
        while k < n:
            rem = n - k
            placed = False
            if rem >= 512 and bs0 <= BUD:
                idxs = [b1[c][k : k + 512] for c in range(NCORES)]
                sp1 = max(
                    float(xs[c, idxs[c], 1].max() - xs[c, idxs[c], 1].min())
                    for c in range(NCORES)
                )
                sp0 = max(
                    float(xs[c, idxs[c], 0].max() - xs[c, idxs[c], 0].min())
                    for c in range(NCORES)
                )
                if sp1 <= BUD and sp0 <= BUD:
                    tiles.append((4, idxs))
                    k += 512
                    placed = True
            if not placed and rem >= 256:
                idxs = [b1[c][k : k + 256] for c in range(NCORES)]
                sp0 = max(
                    float(xs[c, idxs[c], 0].max() - xs[c, idxs[c], 0].min())
                    for c in range(NCORES)
                )
                if sp0 <= BUD:
                    tiles.append((2, idxs))
                    k += 256
                    placed = True
            if not placed:
                tiles.append((1, [b1[c][k : k + 128] for c in range(NCORES)]))
                k += 128

    def win_off(vals_min, vals_max):
        o = int(np.floor((vals_min - REACH - glo) / H))
        o = (o // QUANT) * QUANT
        o = max(0, min(NGR - W, o))
        assert glo + (o + W - 1) * H >= vals_max + REACH - 1e-9
        return o

    sched = []
    for mode, idxs in tiles:
        o0 = o1 = 0
        if mode in (4, 2):
            o0 = win_off(
                min(float(xs[c, idxs[c], 0].min()) for c in range(NCORES)),
                max(float(xs[c, idxs[c], 0].max()) for c in range(NCORES)),
            )
        if mode == 4:
            o1 = win_off(
                min(float(xs[c, idxs[c], 1].min()) for c in range(NCORES)),
                max(float(xs[c, idxs[c], 1].max()) for c in range(NCORES)),
            )
        sched.append((mode, o0, o1, idxs))
    # order: mode-4 blocks, then mode-2, then mode-1 (trailing for mul split)
    sched.sort(key=lambda t: -t[0])
    return sched


def _coef_tile(mode, o0, o1, cb):
    """Host block-diagonal argB lhsT [128,128] bf16 for one distinct."""
    c_hi, c_lo, b_hi, b_lo = cb
    t = np.zeros((128, 128), dtype=np.float64)

    def put(colbase, rowbase, nodes, dim):
        # feature rows within a sub-block: dim0 at rows 0..5, dim1 6..11,
        # ones 12..13
        fr = rowbase + (0 if dim == 0 else 6)
        for ci, j in enumerate(nodes):
            col = colbase + ci
            t[fr + 0, col] = c_hi[j]
            t[fr + 1, col] = c_lo[j]
            t[fr + 2, col] = c_hi[j]
            t[fr + 3, col] = c_lo[j]
            t[fr + 4, col] = -1.0
            t[fr + 5, col] = -1.0
            t[rowbase + 12, col] = b_hi[j]
            t[rowbase + 13, col] = b_lo[j]

    def pad(colbase, rowbase, ncols):
        t[rowbase + 12, colbase : colbase + ncols] = -4900.0

    if mode == 4:
        for s in range(4):
            put(32 * s, 32 * s, range(o0, o0 + W), 0)
            put(32 * s + W, 32 * s, range(o1, o1 + W), 1)
    elif mode == 2:
        for s in range(2):
            put(64 * s, 64 * s, range(o0, o0 + W), 0)
            put(64 * s + W, 64 * s, range(NGR), 1)
    else:
        put(0, 0, range(NGR), 0)
        pad(NGR, 0, NG - NGR)
        put(NG, 0, range(NGR), 1)
        pad(NG + NGR, 0, NG - NGR)
    return t.astype(BF)


def make_in_maps(x, data, weights):
    featd, ga, w0a, o0s, glo, H, u, cb = _host_prep_a(x, data, weights)
    xs = x.reshape(NCORES, NSH, D).astype(np.float64)
    sched = _make_schedule(xs, H, glo)

    # distinct map (sched is mode-desc sorted => d4 ids, then d2, then d1)
    dmap0, did = [], {}
    blocks = []
    for mode, o0, o1, idxs in sched:
        k = (mode, o0, o1)
        if k not in did:
            did[k] = len(dmap0)
            dmap0.append(k)
        blocks.append((mode, did[k]))
    ndist = len(dmap0)

    # selector panels: one per distinct mode-4 o1 value, plus a full-diagonal
    # panel (-1) for mode-2
    pvals = sorted({o1 for (m, o0, o1) in dmap0 if m == 4})
    if any(m == 2 for (m, _, _) in dmap0):
        pvals = pvals + [-1]
    pidx_of = {v: i for i, v in enumerate(pvals)}
    dmap = []
    for m, o0, o1 in dmap0:
        pidx = pidx_of[o1] if m == 4 else (pidx_of[-1] if m == 2 else 0)
        dmap.append((m, o0, o1, pidx))

    global LAST_KEY
    LAST_KEY = (
        int(w0a), tuple(int(v) for v in o0s), tuple(blocks), ndist,
        tuple(dmap), tuple(pvals),
    )

    # featx packing + output gather per core
    NBLK = len(blocks)
    featx = np.zeros((NCORES, 128, NBLK * 128), dtype=np.float64)
    gather = np.zeros((NCORES, NSH), dtype=np.int64)  # loc <- dram index
    cum = 0
    x64 = xs
    for b, (mode, o0, o1, idxs) in enumerate(sched):
        m = mode
        sub = 128 // m
        for c in range(NCORES):
            idx = idxs[c]
            xt = x64[c, idx]  # [nl, 2]
            f0 = np.stack(_feat6(xt[:, 0])).astype(np.float64)  # [6, nl]
            f1 = np.stack(_feat6(xt[:, 1])).astype(np.float64)
            for s in range(m):
                li = np.arange(s * 128, (s + 1) * 128)
                rows = sub * s
                featx[c, rows : rows + 6, b * 128 : (b + 1) * 128] = f0[:, li]
                featx[c, rows + 6 : rows + 12, b * 128 : (b + 1) * 128] = f1[:, li]
                featx[c, rows + 12 : rows + 14, b * 128 : (b + 1) * 128] = 1.0
                # dram index = p*128 + (cum + s); loc = idx[s*128 + p]
                gather[c, idx[li]] = np.arange(128) * 128 + (cum + s)
        cum += m
    assert cum == 128

    gblk = np.zeros((128, ndist * 128), dtype=BF)
    for d, (mode, o0, o1, pidx) in enumerate(dmap):
        gblk[:, d * 128 : (d + 1) * 128] = _coef_tile(mode, o0, o1, cb)

    # selector panels [48, 224] each: panel for o1 value v has
    # panel[j1, 112 + j1 - v] = 1 for j1 in [v, v+W) (v=-1: all 48 rows,
    # diagonal at 112 + j1). Sliced at off = 96 - base so lhsT local col
    # k = base+16+j1m hits j1 = v+j1m.
    selm = np.zeros((48, len(pvals) * 224), dtype=BF)
    for i, v in enumerate(pvals):
        if v < 0:
            for j1 in range(NGR):
                selm[j1, i * 224 + 112 + j1] = 1.0
        else:
            for j1m in range(W):
                selm[v + j1m, i * 224 + 112 + j1m] = 1.0
    onesm = np.zeros((128, 8), dtype=BF)
    for s in range(4):
        onesm[32 * s : 32 * (s + 1), s] = 1.0
    for s in range(2):
        onesm[64 * s : 64 * (s + 1), 4 + s] = 1.0
    onesm[0:NG, 6] = 1.0

    featd_w = _widen(featd, NPTS)
    ga_w = _grep(ga)
    in_maps = []
    for c in range(NCORES):
        in_maps.append({
            "featd": featd_w,
            "ga": ga_w,
            "featx": featx[c].astype(BF),
            "gblk": gblk,
            "selm": selm,
            "onesm": onesm,
        })
    return in_maps, gather


def kernel(x, data, weights):
    global LAST_RESULTS
    x = np.ascontiguousarray(x, dtype=np.float32)
    data = np.ascontiguousarray(data, dtype=np.float32)
    weights = np.ascontiguousarray(weights, dtype=np.float32)
    assert x.shape == (B, L, D) and data.shape == (NPTS, D)

    in_maps, gather = make_in_maps(x, data, weights)
    nc = _get_nc()
    try:
        res = bass_utils.run_bass_kernel_spmd(
            nc, in_maps, core_ids=list(range(NCORES)),
            trace=bool(os.environ.get("BASS_TRACE")),
        )
    except ModuleNotFoundError:
        os.environ["BASS_NEVER_TRACE"] = "1"
        res = bass_utils.run_bass_kernel_spmd(
            nc, in_maps, core_ids=list(range(NCORES)), trace=False,
        )
    LAST_RESULTS = res
    outs = []
    for c in range(NCORES):
        buf = res.results[c]["out"]
        outs.append(buf[gather[c]])
    return np.concatenate(outs).reshape(B, L).astype(np.float32)
